# revision 36
# baseline (speedup 1.0000x reference)
"""Causal self-attention Trainium2 kernel (Bass/Tile), 8-core SPMD.

Problem: X[2, 2048, 1024], W_qkv[1024, 3072], W_proj[1024, 1024], H=16 heads.

Sharding: core c handles batch b = c // 4 and heads h0 = 4*(c % 4) .. h0+4
(tensor-parallel over heads + data-parallel over batch). Each core computes
a partial output  out_b = Y[:, heads] @ W_proj[head rows, :]  and the host
sums the 4 partials per batch (the W_proj row-shard reduction).

Per-core device layout ("transposed attention", no P transposes needed):
  Xt  [C, T]      X[b].T, all 32 [128, 512] tiles DMAed up front
  Qt,Kt [128,2,CH] per chunk, per head-pair group g: partition = 64*(h%2)+d
  V   [128,4,260] per chunk [token-block, head*65(+ones col)] for PV lhsT
  St  = Kt_blk.T @ Qt_chunk -> [keys 128, q 512] PSUM (K=d=64 contraction),
        both heads of a pair packed side-by-side in one [128, 1024] tile
  P   = exp(0.125*(St + causal_mask))  via ACT; ones-augmented PV gives
  Yt_aug = [V|1].T @ P -> [65, q 512]: rows 0-63 = Yt, row 64 = softmax sums

Scheduling: the TRN2 PE clock ramps 0.65 -> 1.2 -> 2.4 GHz with ~3us of
CONTINUOUS execution and drops back on idle, so the whole kernel is emitted
as one dense PE stream: warmup matmuls cover the initial weight DMAs, and
the QKV projection for token chunk tch+1 plus the output projection for
chunk qi-1 are injected one group per key-block iteration into the
attention loop over chunk qi, so the PE never starves while ACT runs exp.
ST matmuls and exp skip the fully-masked columns left of the causal
diagonal; PV reads only the surviving columns.
"""

import numpy as np
from collections import deque

B, T, C, H = 2, 2048, 1024, 16
HD = 64          # head dim
HPC = 4          # heads per core
P = 128
NCORES = 8
CH = 512         # token chunk (matmul free dim / q chunk)
KB = 128         # key block
MASK_VAL = -1.0e5
MM_DTYPE = "bf16"


def build_nc(t_len=T, mm_dtype=None):
    import concourse.bass as bass
    import concourse.mybir as mybir
    from concourse import bacc, library_config
    from concourse.tile import TileContext
    from contextlib import ExitStack

    mm_dtype = mm_dtype or MM_DTYPE
    f32 = mybir.dt.float32
    mdt = mybir.dt.bfloat16 if mm_dtype == "bf16" else mybir.dt.float32r
    Exp = mybir.ActivationFunctionType.Exp
    Alu = mybir.AluOpType

    NKC = C // P          # 8 contraction chunks over C
    NCH = t_len // CH     # token chunks
    NBC = CH // P         # token blocks per chunk (4)
    SC = 1.0 / np.sqrt(HD)

    nc = bacc.Bacc("TRN2", target_bir_lowering=False, debug=False,
                   num_devices=NCORES)

    xt_d = nc.dram_tensor("xt", [C, t_len], mdt, kind="ExternalInput").ap()
    wq_d = nc.dram_tensor("wq", [P, NKC, HPC * HD], mdt, kind="ExternalInput").ap()
    wk_d = nc.dram_tensor("wk", [P, NKC, HPC * HD], mdt, kind="ExternalInput").ap()
    wv_d = nc.dram_tensor("wv", [P, NKC, HPC * HD], mdt, kind="ExternalInput").ap()
    wp_d = nc.dram_tensor("wp", [P, 2, C], mdt, kind="ExternalInput").ap()
    out_d = nc.dram_tensor("out", [t_len, C], mdt, kind="ExternalOutput").ap()

    with TileContext(nc) as tc, ExitStack() as ctx:
        const = ctx.enter_context(tc.tile_pool(name="const", bufs=1))
        work = ctx.enter_context(tc.tile_pool(name="work", bufs=3))
        psm = ctx.enter_context(tc.tile_pool(name="psm", bufs=2, space="PSUM"))

        # ---- persistent SBUF tensors ----
        wq_sb = const.tile([P, NKC, HPC * HD], mdt, tag="wq")
        wk_sb = const.tile([P, NKC, HPC * HD], mdt, tag="wk")
        wv_sb = const.tile([P, NKC, HPC * HD], mdt, tag="wv")
        wp_sb = const.tile([P, 2, C], mdt, tag="wp")
        # per-chunk tensors (separate tiles -> no false cross-chunk deps)
        qt = [const.tile([P, 2, CH], mdt, tag=f"qt{t}", name=f"qt{t}")
              for t in range(NCH)]
        kt = [const.tile([P, 2, CH], mdt, tag=f"kt{t}", name=f"kt{t}")
              for t in range(NCH)]
        va = [const.tile([P, NBC, HPC * (HD + 1)], mdt, tag=f"va{t}",
                         name=f"va{t}") for t in range(NCH)]
        y2 = [const.tile([P, 2, CH], mdt, tag=f"y2{t}", name=f"y2{t}")
              for t in range(NCH)]
        xts = [[const.tile([P, CH], mdt, tag=f"xt{t}_{kc}", name=f"xt{t}_{kc}")
                for kc in range(NKC)] for t in range(NCH)]

        def xsrc(tch, kc, c0=0, c1=CH):
            return xts[tch][kc][:, c0:c1]
        warm = const.tile([P, CH], mdt, tag="warm")

        # ---- PE warmup: dense junk matmuls so the tensor engine p-state
        # ramps while the first weight/activation DMAs are in flight.  The
        # memset is the FIRST emitted instruction so nothing can precede it
        # in the DVE queue and stall the PE stream behind it. ----
        nc.vector.memset(warm[:], 0.0)
        wt = psm.tile([P, 2 * CH], f32, tag="st", bufs=3, name="warm")[:, :CH]
        NW = 16
        for i in range(NW):
            nc.tensor.matmul(wt[:], lhsT=warm[:, :P], rhs=warm[:],
                             start=(i == 0), stop=(i == NW - 1))
        nc.gpsimd.load_library(library_config.proxy)

        # ---- input DMAs: wq + chunk-0 Xt first so QKV(0) can start; wk/wv
        # issued early (Sync issues DMAs serially at ~0.6us each and the
        # k/v matmul groups need them before the Xt tail) ----
        nc.sync.dma_start(wq_sb[:, :NKC // 2], wq_d[:, :NKC // 2])
        nc.sync.dma_start(xts[0][0][:], xt_d[0:P, 0:CH])
        nc.sync.dma_start(wq_sb[:, NKC // 2:], wq_d[:, NKC // 2:])
        nc.sync.dma_start(xts[0][1][:], xt_d[P:2 * P, 0:CH])
        nc.sync.dma_start(wk_sb[:, :NKC // 2], wk_d[:, :NKC // 2])
        nc.sync.dma_start(wk_sb[:, NKC // 2:], wk_d[:, NKC // 2:])
        for kc in range(2, NKC):
            nc.sync.dma_start(xts[0][kc][:], xt_d[kc * P:(kc + 1) * P, 0:CH])
        nc.sync.dma_start(wv_sb[:, :NKC // 2], wv_d[:, :NKC // 2])
        nc.sync.dma_start(wv_sb[:, NKC // 2:], wv_d[:, NKC // 2:])
        nc.sync.dma_start(wp_sb[:], wp_d[:])
        for tch in range(1, NCH):
            for kc in range(NKC):
                nc.sync.dma_start(
                    xts[tch][kc][:],
                    xt_d[kc * P:(kc + 1) * P, tch * CH:(tch + 1) * CH])

        # ones columns of the augmented V (softmax denominator trick).  The
        # ones column comes FIRST in each head's 65-block so the PV sums row
        # lands in PSUM partition 0, where partition_broadcast can read it
        # without a bounce DMA.
        for t in range(NCH):
            for hl in range(HPC):
                nc.vector.memset(
                    va[t][:, :, hl * (HD + 1): hl * (HD + 1) + 1], 1.0)

        # ---- QKV projection for one token chunk: 8 injectable PE groups ----
        def qkv_group_thunks(tch):
            thunks = []

            def qk_group(w_sb, dst, g):
                def run():
                    pq = psm.tile([P, 2 * CH], f32, tag="st", bufs=3,
                                  name="pq")[:, :CH]
                    for kc in range(NKC):
                        nc.tensor.matmul(
                            pq[:],
                            lhsT=w_sb[:, kc, g * P:(g + 1) * P],
                            rhs=xsrc(tch, kc),
                            start=(kc == 0), stop=(kc == NKC - 1))
                    nc.vector.tensor_copy(out=dst[tch][:, g, :], in_=pq[:])
                return run

            def v_group(vb):
                def run():
                    pv = psm.tile([P, 2 * CH], f32, tag="st", bufs=3,
                                  name="pv")[:, :HPC * HD]
                    for kc in range(NKC):
                        nc.tensor.matmul(
                            pv,
                            lhsT=xsrc(tch, kc, vb * P, (vb + 1) * P),
                            rhs=wv_sb[:, kc, :],
                            start=(kc == 0), stop=(kc == NKC - 1))
                    nc.vector.tensor_copy(
                        out=va[tch][:, vb, :]
                        .rearrange("p (h e) -> p h e", e=HD + 1)[:, :, 1:],
                        in_=pv.rearrange("p (h e) -> p h e", e=HD))
                return run

            for g in range(2):
                thunks.append(qk_group(wq_sb, qt, g))
                thunks.append(qk_group(wk_sb, kt, g))
            for vb in range(NBC):
                thunks.append(v_group(vb))
            return thunks

        # ---- output projection for one chunk: 4 injectable PE blocks,
        # each computing a full [128, 1024] token block into the two banks
        # of one PSUM slot, drained by one copy and one merged DMA ----
        def proj_pair_thunks(qi):
            thunks = []

            def block(tbl):
                def run():
                    pp = psm.tile([P, 2 * CH], f32, tag="st", bufs=3,
                                  name="pp")
                    for oc in range(2):
                        for yc in range(2):
                            nc.tensor.matmul(
                                pp[:, oc * CH:(oc + 1) * CH],
                                lhsT=y2[qi][:, yc, tbl * P:(tbl + 1) * P],
                                rhs=wp_sb[:, yc, oc * CH:(oc + 1) * CH],
                                start=(yc == 0), stop=(yc == 1))
                    ost = work.tile([P, 2 * CH], mdt, tag="ost", bufs=3)
                    nc.vector.tensor_copy(out=ost[:], in_=pp[:])
                    tb = qi * NBC + tbl
                    nc.sync.dma_start(out_d[tb * P:(tb + 1) * P, :], ost[:])
                return run

            for tbl in range(NBC):
                thunks.append(block(tbl))
            return thunks

        # ---- normalize Yt rows by the sums rows (ytp partition 0), store
        # into y2 via partition-shifting SBUF->SBUF DMAs.  Both heads of the
        # pair share one broadcast/reciprocal/multiply over [65, 2*CH]. ----
        def normalize_pair(g, qi, ytps, last=False):
            """The PSUM tiles are drained immediately (copies) so the pool
            slots free before the reciprocal chain completes.  On the final
            chunk the hh=1 copy moves to ACT (idle by then) so the two
            drains don't serialize on DVE ahead of the last proj."""
            bnc = work.tile([HD + 1, 2, CH], f32, tag="bounce", bufs=2)
            nc.vector.tensor_copy(out=bnc[:, 0, :], in_=ytps[0][:])
            if last:
                nc.scalar.copy(out=bnc[:, 1, :], in_=ytps[1][:])
            else:
                nc.vector.tensor_copy(out=bnc[:, 1, :], in_=ytps[1][:])
            # partition_broadcast / custom-DVE ops ignore the AP base
            # partition on HW; the sums rows are at partition 0 by layout.
            rb = work.tile([HD + 1, 2, CH], f32, tag="rb", bufs=2)
            nc.gpsimd.partition_broadcast(rb[:], bnc[0:1, :, :])
            nc.vector.reciprocal_approx_fast(out=rb[:], in_=rb[:])
            ybs = work.tile([HD + 1, 2, CH], mdt, tag="ybs", bufs=2)
            # row 0 computes sums * 1/sums; only rows 1..64 are stored.
            # (DVE partition base must be aligned, so operate on [0:65].)
            nc.vector.tensor_tensor(out=ybs[:], in0=bnc[:], in1=rb[:],
                                    op=Alu.mult)
            for hh in range(2):
                # shift each head's 64 rows to partitions hh*64 .. hh*64+63
                nc.sync.dma_start(y2[qi][hh * HD:(hh + 1) * HD, g, :],
                                  ybs[1:, hh, :])

        # ---- attention for one query chunk, with PE work injection ----
        def attention(qi, queue, start_at=0):
            nkb = NBC * (qi + 1)
            it = 0
            for g in range(2):
                ytps = [psm.tile([HD + 1, CH], f32, tag="yt", bufs=2,
                                 name=f"ytp{hh}") for hh in range(2)]

                def emit_pv(pts, kb, qoff):
                    for hh in range(2):
                        hl = 2 * g + hh
                        nc.tensor.matmul(
                            ytps[hh][:, qoff:],
                            lhsT=va[kb // NBC][:, kb % NBC,
                                               hl * (HD + 1):(hl + 1) * (HD + 1)],
                            rhs=pts[:, hh * CH + qoff:(hh + 1) * CH],
                            start=(kb == 0), stop=(kb == nkb - 1))

                prev = None
                for kb in range(nkb):
                    o4 = kb - NBC * qi          # 0..3 on the diagonal band
                    qoff = max(0, o4) * KB
                    stp = psm.tile([P, 2 * CH], f32, tag="st", bufs=3,
                                   name="stp")
                    for hh in range(2):
                        nc.tensor.matmul(
                            stp[:, hh * CH + qoff:(hh + 1) * CH],
                            lhsT=kt[kb // NBC][hh * HD:(hh + 1) * HD, g,
                                               (kb % NBC) * KB:(kb % NBC + 1) * KB],
                            rhs=qt[qi][hh * HD:(hh + 1) * HD, g, qoff:],
                            start=True, stop=True)
                    # PV of the previous block slots in while this block's
                    # exp runs, keeping PE fed (software pipelining).
                    if prev is not None:
                        emit_pv(*prev)
                    if queue and it >= start_at:
                        n = -(-len(queue) // max(1, 2 * nkb - it))
                        for _ in range(min(n, len(queue))):
                            queue.popleft()()
                    it += 1
                    pts = work.tile([P, 2 * CH], mdt, tag="p", bufs=4,
                                    name="pts")
                    if o4 >= 0:
                        nc.scalar.activation(
                            out=pts.rearrange("p (h c) -> p h c", h=2)[:, :, qoff:],
                            in_=stp.rearrange("p (h c) -> p h c", h=2)[:, :, qoff:],
                            func=Exp, scale=SC)
                        # causal mask: zero the upper triangle of the
                        # 128-wide diagonal band of P (keep where q >= k,
                        # i.e. band column f >= partition p).  Runs on
                        # GpSimd so the DVE queue never delays exp or PV.
                        for hh in range(2):
                            nc.gpsimd.affine_select(
                                out=pts[:, hh * CH + qoff:hh * CH + qoff + KB],
                                in_=pts[:, hh * CH + qoff:hh * CH + qoff + KB],
                                compare_op=Alu.is_ge, fill=0.0,
                                base=0, channel_multiplier=-1,
                                pattern=[[1, KB]])
                    else:
                        nc.scalar.activation(out=pts[:], in_=stp[:],
                                             func=Exp, scale=SC)
                    prev = (pts, kb, qoff)
                emit_pv(*prev)
                normalize_pair(g, qi, ytps,
                               last=(qi == NCH - 1 and g == 1))

        # ---- emission schedule ----
        # phase A: only q-g0/k-g0 of chunk 0 up front; the v-groups and the
        # g1 projections are injected into attention(0) just ahead of need.
        t0 = qkv_group_thunks(0)
        for th in t0[:2]:
            th()
        for qi in range(NCH):
            queue = deque()
            if qi == 0:
                queue.extend([t0[4], t0[5], t0[2], t0[3], t0[6], t0[7]])
            if qi + 1 < NCH:
                queue.extend(qkv_group_thunks(qi + 1))
            start_at = 0
            if qi > 0:
                queue.extend(proj_pair_thunks(qi - 1))
                start_at = 3 if qi == NCH - 1 else 0
            attention(qi, queue, start_at)
            while queue:
                queue.popleft()()
        # bridge: junk matmuls keep the PE p-state at max while the last
        # chunk's normalize chains run, so the final proj starts hot and
        # immediately.
        wt2 = psm.tile([P, 2 * CH], f32, tag="st", bufs=3,
                       name="bridge")[:, :CH]
        NB = 34
        for i in range(NB):
            nc.tensor.matmul(wt2[:], lhsT=warm[:, :P], rhs=warm[:],
                             start=(i == 0), stop=(i == NB - 1))
        for th in proj_pair_thunks(NCH - 1):
            th()
    nc.compile()
    return nc


def _to_mm_dtype(a):
    if MM_DTYPE == "bf16":
        import ml_dtypes
        return np.ascontiguousarray(a).astype(ml_dtypes.bfloat16)
    return np.ascontiguousarray(a).astype(np.float32)


def make_in_maps(X, W_qkv, W_proj, t_len=T):
    """Host-side sharding: slice + pre-arrange weights per head group,
    transpose X.  Layouts match the SBUF tensors so every weight DMA is
    fully contiguous:
      wq/wk/wv [128, 8, 256]: [p, kc, m] = W[kc*128+p, cols][m]
      wp       [128, 2, C]:   [64*hh+d, yc, m] = W_proj[(2*yc+hh)*64+d, m]
    """
    in_maps = []
    xts = [_to_mm_dtype(np.asarray(X[b, :t_len, :]).T) for b in range(B)]
    NKC = C // P
    for c in range(NCORES):
        b = c // (NCORES // B)
        h0 = HPC * (c % (NCORES // B))
        cols = slice(h0 * HD, (h0 + HPC) * HD)

        def warr(w):
            return _to_mm_dtype(
                np.ascontiguousarray(w).reshape(NKC, P, HPC * HD)
                .transpose(1, 0, 2))

        wp_c = np.ascontiguousarray(W_proj[cols, :])          # [256, C]
        wp2 = wp_c.reshape(2, 2, HD, C).transpose(1, 2, 0, 3).reshape(P, 2, C)
        in_maps.append({
            "xt": xts[b],
            "wq": warr(W_qkv[:, cols]),
            "wk": warr(W_qkv[:, C:][:, cols]),
            "wv": warr(W_qkv[:, 2 * C:][:, cols]),
            "wp": _to_mm_dtype(wp2),
        })
    return in_maps


_CACHE = {}
TRACE = False           # set True (e.g. from test.py) to capture an NTFF profile


def kernel(X, W_qkv, W_proj):
    import sys
    if "/opt/trn_rl_repo" not in sys.path:
        sys.path.insert(0, "/opt/trn_rl_repo")
    from concourse.bass_utils import run_bass_kernel_spmd

    X = np.asarray(X, dtype=np.float32)
    W_qkv = np.asarray(W_qkv, dtype=np.float32)
    W_proj = np.asarray(W_proj, dtype=np.float32)

    if "nc" not in _CACHE:
        _CACHE["nc"] = build_nc()
    nc = _CACHE["nc"]

    in_maps = make_in_maps(X, W_qkv, W_proj)
    res = run_bass_kernel_spmd(nc, in_maps, core_ids=list(range(NCORES)),
                               trace=TRACE)
    _CACHE["last"] = res
    out = np.empty((B, T, C), dtype=np.float32)
    ncb = NCORES // B
    for b in range(B):
        acc = res.results[b * ncb]["out"].astype(np.float32)
        for c in range(b * ncb + 1, (b + 1) * ncb):
            acc = acc + res.results[c]["out"].astype(np.float32)
        out[b] = acc
    return out


# revision 37
# speedup vs baseline: 1.1370x; 1.1370x over previous
"""Causal self-attention Trainium2 kernel (Bass/Tile), 8-core SPMD.

Problem: X[2, 2048, 1024], W_qkv[1024, 3072], W_proj[1024, 1024], H=16 heads.

Sharding: core c handles batch b = c // 4 and heads h0 = 4*(c % 4) .. h0+4
(tensor-parallel over heads + data-parallel over batch). Each core computes
a partial output  out_b = Y[:, heads] @ W_proj[head rows, :]  and the host
sums the 4 partials per batch (the W_proj row-shard reduction).

Per-core device layout ("transposed attention", no P transposes needed):
  Xt  [C, T]      X[b].T, all 32 [128, 512] tiles DMAed up front
  Qt,Kt [128,2,CH] per chunk, per head-pair group g: partition = 64*(h%2)+d
  V   [128,4,260] per chunk [token-block, head*65(+ones col)] for PV lhsT
  St  = Kt_blk.T @ Qt_chunk -> [keys 128, q 512] PSUM (K=d=64 contraction),
        both heads of a pair packed side-by-side in one [128, 1024] tile
  P   = exp(0.125*(St + causal_mask))  via ACT; ones-augmented PV gives
  Yt_aug = [V|1].T @ P -> [65, q 512]: rows 0-63 = Yt, row 64 = softmax sums

Scheduling: the TRN2 PE clock ramps 0.65 -> 1.2 -> 2.4 GHz with ~3us of
CONTINUOUS execution and drops back on idle, so the whole kernel is emitted
as one dense PE stream: warmup matmuls cover the initial weight DMAs, and
the QKV projection for token chunk tch+1 plus the output projection for
chunk qi-1 are injected one group per key-block iteration into the
attention loop over chunk qi, so the PE never starves while ACT runs exp.
ST matmuls and exp skip the fully-masked columns left of the causal
diagonal; PV reads only the surviving columns.
"""

import numpy as np
from collections import deque

B, T, C, H = 2, 2048, 1024, 16
HD = 64          # head dim
HPC = 4          # heads per core
P = 128
NCORES = 8
CH = 512         # token chunk (matmul free dim / q chunk)
KB = 128         # key block
MASK_VAL = -1.0e5
MM_DTYPE = "bf16"


def build_nc(t_len=T, mm_dtype=None):
    import concourse.bass as bass
    import concourse.mybir as mybir
    from concourse import bacc, library_config
    from concourse.tile import TileContext
    from contextlib import ExitStack

    mm_dtype = mm_dtype or MM_DTYPE
    f32 = mybir.dt.float32
    mdt = mybir.dt.bfloat16 if mm_dtype == "bf16" else mybir.dt.float32r
    Exp = mybir.ActivationFunctionType.Exp
    Alu = mybir.AluOpType

    NKC = C // P          # 8 contraction chunks over C
    NCH = t_len // CH     # token chunks
    NBC = CH // P         # token blocks per chunk (4)
    SC = 1.0 / np.sqrt(HD)

    nc = bacc.Bacc("TRN2", target_bir_lowering=False, debug=False,
                   num_devices=NCORES)

    xt_d = nc.dram_tensor("xt", [C, t_len], mdt, kind="ExternalInput").ap()
    wq_d = nc.dram_tensor("wq", [P, NKC, HPC * HD], mdt, kind="ExternalInput").ap()
    wk_d = nc.dram_tensor("wk", [P, NKC, HPC * HD], mdt, kind="ExternalInput").ap()
    wv_d = nc.dram_tensor("wv", [P, NKC, HPC * HD], mdt, kind="ExternalInput").ap()
    wp_d = nc.dram_tensor("wp", [P, 2, C], mdt, kind="ExternalInput").ap()
    out_d = nc.dram_tensor("out", [t_len, C], mdt, kind="ExternalOutput").ap()

    with TileContext(nc) as tc, ExitStack() as ctx:
        const = ctx.enter_context(tc.tile_pool(name="const", bufs=1))
        work = ctx.enter_context(tc.tile_pool(name="work", bufs=3))
        psm = ctx.enter_context(tc.tile_pool(name="psm", bufs=2, space="PSUM"))

        # ---- persistent SBUF tensors ----
        wq_sb = const.tile([P, NKC, HPC * HD], mdt, tag="wq")
        wk_sb = const.tile([P, NKC, HPC * HD], mdt, tag="wk")
        wv_sb = const.tile([P, NKC, HPC * HD], mdt, tag="wv")
        wp_sb = const.tile([P, 2, C], mdt, tag="wp")
        # per-chunk tensors (separate tiles -> no false cross-chunk deps)
        qt = [const.tile([P, 2, CH], mdt, tag=f"qt{t}", name=f"qt{t}")
              for t in range(NCH)]
        kt = [const.tile([P, 2, CH], mdt, tag=f"kt{t}", name=f"kt{t}")
              for t in range(NCH)]
        va = [const.tile([P, NBC, HPC * (HD + 1)], mdt, tag=f"va{t}",
                         name=f"va{t}") for t in range(NCH)]
        y2 = [const.tile([P, 2, CH], mdt, tag=f"y2{t}", name=f"y2{t}")
              for t in range(NCH)]
        xts = [[const.tile([P, CH], mdt, tag=f"xt{t}_{kc}", name=f"xt{t}_{kc}")
                for kc in range(NKC)] for t in range(NCH)]

        def xsrc(tch, kc, c0=0, c1=CH):
            return xts[tch][kc][:, c0:c1]
        warm = const.tile([P, CH], mdt, tag="warm")

        # ---- PE warmup: dense junk matmuls so the tensor engine p-state
        # ramps while the first weight/activation DMAs are in flight.  The
        # memset is the FIRST emitted instruction so nothing can precede it
        # in the DVE queue and stall the PE stream behind it. ----
        nc.vector.memset(warm[:], 0.0)
        wt = psm.tile([P, 2 * CH], f32, tag="st", bufs=3, name="warm")[:, :CH]
        NW = 16
        for i in range(NW):
            nc.tensor.matmul(wt[:], lhsT=warm[:, :P], rhs=warm[:],
                             start=(i == 0), stop=(i == NW - 1))
        nc.gpsimd.load_library(library_config.proxy)

        # ---- input DMAs: wq + chunk-0 Xt first so QKV(0) can start; wk/wv
        # issued early (Sync issues DMAs serially at ~0.6us each and the
        # k/v matmul groups need them before the Xt tail) ----
        nc.sync.dma_start(wq_sb[:, :NKC // 2], wq_d[:, :NKC // 2])
        nc.sync.dma_start(xts[0][0][:], xt_d[0:P, 0:CH])
        nc.sync.dma_start(wq_sb[:, NKC // 2:], wq_d[:, NKC // 2:])
        nc.sync.dma_start(xts[0][1][:], xt_d[P:2 * P, 0:CH])
        nc.sync.dma_start(wk_sb[:, :NKC // 2], wk_d[:, :NKC // 2])
        nc.sync.dma_start(wk_sb[:, NKC // 2:], wk_d[:, NKC // 2:])
        for kc in range(2, NKC):
            nc.sync.dma_start(xts[0][kc][:], xt_d[kc * P:(kc + 1) * P, 0:CH])
        nc.sync.dma_start(wv_sb[:, :NKC // 2], wv_d[:, :NKC // 2])
        nc.sync.dma_start(wv_sb[:, NKC // 2:], wv_d[:, NKC // 2:])
        nc.sync.dma_start(wp_sb[:], wp_d[:])
        for tch in range(1, NCH):
            for kc in range(NKC):
                nc.sync.dma_start(
                    xts[tch][kc][:],
                    xt_d[kc * P:(kc + 1) * P, tch * CH:(tch + 1) * CH])

        # ones columns of the augmented V (softmax denominator trick).  The
        # ones column comes FIRST in each head's 65-block so the PV sums row
        # lands in PSUM partition 0, where partition_broadcast can read it
        # without a bounce DMA.
        for t in range(NCH):
            for hl in range(HPC):
                nc.vector.memset(
                    va[t][:, :, hl * (HD + 1): hl * (HD + 1) + 1], 1.0)

        # ---- QKV projection for one token chunk: 8 injectable PE groups ----
        def qkv_group_thunks(tch):
            thunks = []

            def qk_group(w_sb, dst, g):
                def run():
                    pq = psm.tile([P, 2 * CH], f32, tag="st", bufs=3,
                                  name="pq")[:, :CH]
                    for kc in range(NKC):
                        nc.tensor.matmul(
                            pq[:],
                            lhsT=w_sb[:, kc, g * P:(g + 1) * P],
                            rhs=xsrc(tch, kc),
                            start=(kc == 0), stop=(kc == NKC - 1))
                    nc.vector.tensor_copy(out=dst[tch][:, g, :], in_=pq[:])
                return run

            def v_group(vb):
                def run():
                    pv = psm.tile([P, 2 * CH], f32, tag="st", bufs=3,
                                  name="pv")[:, :HPC * HD]
                    for kc in range(NKC):
                        nc.tensor.matmul(
                            pv,
                            lhsT=xsrc(tch, kc, vb * P, (vb + 1) * P),
                            rhs=wv_sb[:, kc, :],
                            start=(kc == 0), stop=(kc == NKC - 1))
                    nc.vector.tensor_copy(
                        out=va[tch][:, vb, :]
                        .rearrange("p (h e) -> p h e", e=HD + 1)[:, :, 1:],
                        in_=pv.rearrange("p (h e) -> p h e", e=HD))
                return run

            for g in range(2):
                thunks.append(qk_group(wq_sb, qt, g))
                thunks.append(qk_group(wk_sb, kt, g))
            for vb in range(NBC):
                thunks.append(v_group(vb))
            return thunks

        # ---- output projection for one chunk: 8 injectable PE pairs ----
        def proj_pair_thunks(qi):
            thunks = []

            def pair(tbl, oc):
                def run():
                    pp = psm.tile([P, 2 * CH], f32, tag="st", bufs=3,
                                  name="pp")[:, :CH]
                    for yc in range(2):
                        nc.tensor.matmul(
                            pp[:],
                            lhsT=y2[qi][:, yc, tbl * P:(tbl + 1) * P],
                            rhs=wp_sb[:, yc, oc * CH:(oc + 1) * CH],
                            start=(yc == 0), stop=(yc == 1))
                    ost = work.tile([P, CH], mdt, tag="ost", bufs=3)
                    nc.vector.tensor_copy(out=ost[:], in_=pp[:])
                    tb = qi * NBC + tbl
                    nc.sync.dma_start(
                        out_d[tb * P:(tb + 1) * P, oc * CH:(oc + 1) * CH],
                        ost[:])
                return run

            for tbl in range(NBC):
                for oc in range(2):
                    thunks.append(pair(tbl, oc))
            return thunks

        # ---- normalize Yt rows by the sums rows (ytp partition 0), store
        # into y2 via partition-shifting SBUF->SBUF DMAs.  Both heads of the
        # pair share one broadcast/reciprocal/multiply over [65, 2*CH]. ----
        def normalize_pair(g, qi, ytps, last=False):
            """The PSUM tiles are drained immediately (copies) so the pool
            slots free before the reciprocal chain completes.  On the final
            chunk the hh=1 copy moves to ACT (idle by then) so the two
            drains don't serialize on DVE ahead of the last proj."""
            bnc = work.tile([HD + 1, 2, CH], f32, tag="bounce", bufs=2)
            nc.vector.tensor_copy(out=bnc[:, 0, :], in_=ytps[0][:])
            if last:
                nc.scalar.copy(out=bnc[:, 1, :], in_=ytps[1][:])
            else:
                nc.vector.tensor_copy(out=bnc[:, 1, :], in_=ytps[1][:])
            # partition_broadcast / custom-DVE ops ignore the AP base
            # partition on HW; the sums rows are at partition 0 by layout.
            rb = work.tile([HD + 1, 2, CH], f32, tag="rb", bufs=2)
            nc.gpsimd.partition_broadcast(rb[:], bnc[0:1, :, :])
            nc.vector.reciprocal_approx_fast(out=rb[:], in_=rb[:])
            ybs = work.tile([HD + 1, 2, CH], mdt, tag="ybs", bufs=2)
            # row 0 computes sums * 1/sums; only rows 1..64 are stored.
            # (DVE partition base must be aligned, so operate on [0:65].)
            nc.vector.tensor_tensor(out=ybs[:], in0=bnc[:], in1=rb[:],
                                    op=Alu.mult)
            for hh in range(2):
                # shift each head's 64 rows to partitions hh*64 .. hh*64+63
                nc.sync.dma_start(y2[qi][hh * HD:(hh + 1) * HD, g, :],
                                  ybs[1:, hh, :])

        # ---- attention for one query chunk, with PE work injection ----
        def attention(qi, queue, start_at=0):
            nkb = NBC * (qi + 1)
            it = 0
            for g in range(2):
                ytps = [psm.tile([HD + 1, CH], f32, tag="yt", bufs=2,
                                 name=f"ytp{hh}") for hh in range(2)]

                def emit_pv(pts, kb, qoff):
                    for hh in range(2):
                        hl = 2 * g + hh
                        nc.tensor.matmul(
                            ytps[hh][:, qoff:],
                            lhsT=va[kb // NBC][:, kb % NBC,
                                               hl * (HD + 1):(hl + 1) * (HD + 1)],
                            rhs=pts[:, hh * CH + qoff:(hh + 1) * CH],
                            start=(kb == 0), stop=(kb == nkb - 1))

                prev = None
                for kb in range(nkb):
                    o4 = kb - NBC * qi          # 0..3 on the diagonal band
                    qoff = max(0, o4) * KB
                    stp = psm.tile([P, 2 * CH], f32, tag="st", bufs=3,
                                   name="stp")
                    for hh in range(2):
                        nc.tensor.matmul(
                            stp[:, hh * CH + qoff:(hh + 1) * CH],
                            lhsT=kt[kb // NBC][hh * HD:(hh + 1) * HD, g,
                                               (kb % NBC) * KB:(kb % NBC + 1) * KB],
                            rhs=qt[qi][hh * HD:(hh + 1) * HD, g, qoff:],
                            start=True, stop=True)
                    # PV of the previous block slots in while this block's
                    # exp runs, keeping PE fed (software pipelining).
                    if prev is not None:
                        emit_pv(*prev)
                    if queue and it >= start_at:
                        n = -(-len(queue) // max(1, 2 * nkb - it))
                        for _ in range(min(n, len(queue))):
                            queue.popleft()()
                    it += 1
                    pts = work.tile([P, 2 * CH], mdt, tag="p", bufs=4,
                                    name="pts")
                    if o4 >= 0:
                        nc.scalar.activation(
                            out=pts.rearrange("p (h c) -> p h c", h=2)[:, :, qoff:],
                            in_=stp.rearrange("p (h c) -> p h c", h=2)[:, :, qoff:],
                            func=Exp, scale=SC)
                        # causal mask: zero the upper triangle of the
                        # 128-wide diagonal band of P (keep where q >= k,
                        # i.e. band column f >= partition p).  Runs on
                        # GpSimd so the DVE queue never delays exp or PV.
                        for hh in range(2):
                            nc.gpsimd.affine_select(
                                out=pts[:, hh * CH + qoff:hh * CH + qoff + KB],
                                in_=pts[:, hh * CH + qoff:hh * CH + qoff + KB],
                                compare_op=Alu.is_ge, fill=0.0,
                                base=0, channel_multiplier=-1,
                                pattern=[[1, KB]])
                    else:
                        nc.scalar.activation(out=pts[:], in_=stp[:],
                                             func=Exp, scale=SC)
                    prev = (pts, kb, qoff)
                emit_pv(*prev)
                normalize_pair(g, qi, ytps,
                               last=(qi == NCH - 1 and g == 1))

        # ---- emission schedule ----
        # phase A: only q-g0/k-g0 of chunk 0 up front; the v-groups and the
        # g1 projections are injected into attention(0) just ahead of need.
        t0 = qkv_group_thunks(0)
        for th in t0[:2]:
            th()
        for qi in range(NCH):
            queue = deque()
            if qi == 0:
                queue.extend([t0[4], t0[5], t0[2], t0[3], t0[6], t0[7]])
            if qi + 1 < NCH:
                queue.extend(qkv_group_thunks(qi + 1))
            start_at = 0
            if qi > 0:
                queue.extend(proj_pair_thunks(qi - 1))
                start_at = 3 if qi == NCH - 1 else 0
            attention(qi, queue, start_at)
            while queue:
                queue.popleft()()
        # bridge: junk matmuls keep the PE p-state at max while the last
        # chunk's normalize chains run, so the final proj starts hot and
        # immediately.
        wt2 = psm.tile([P, 2 * CH], f32, tag="st", bufs=3,
                       name="bridge")[:, :CH]
        NB = 34
        for i in range(NB):
            nc.tensor.matmul(wt2[:], lhsT=warm[:, :P], rhs=warm[:],
                             start=(i == 0), stop=(i == NB - 1))
        for th in proj_pair_thunks(NCH - 1):
            th()
    nc.compile()
    return nc


def _to_mm_dtype(a):
    if MM_DTYPE == "bf16":
        import ml_dtypes
        return np.ascontiguousarray(a).astype(ml_dtypes.bfloat16)
    return np.ascontiguousarray(a).astype(np.float32)


def make_in_maps(X, W_qkv, W_proj, t_len=T):
    """Host-side sharding: slice + pre-arrange weights per head group,
    transpose X.  Layouts match the SBUF tensors so every weight DMA is
    fully contiguous:
      wq/wk/wv [128, 8, 256]: [p, kc, m] = W[kc*128+p, cols][m]
      wp       [128, 2, C]:   [64*hh+d, yc, m] = W_proj[(2*yc+hh)*64+d, m]
    """
    in_maps = []
    xts = [_to_mm_dtype(np.asarray(X[b, :t_len, :]).T) for b in range(B)]
    NKC = C // P
    for c in range(NCORES):
        b = c // (NCORES // B)
        h0 = HPC * (c % (NCORES // B))
        cols = slice(h0 * HD, (h0 + HPC) * HD)

        def warr(w):
            return _to_mm_dtype(
                np.ascontiguousarray(w).reshape(NKC, P, HPC * HD)
                .transpose(1, 0, 2))

        wp_c = np.ascontiguousarray(W_proj[cols, :])          # [256, C]
        wp2 = wp_c.reshape(2, 2, HD, C).transpose(1, 2, 0, 3).reshape(P, 2, C)
        in_maps.append({
            "xt": xts[b],
            "wq": warr(W_qkv[:, cols]),
            "wk": warr(W_qkv[:, C:][:, cols]),
            "wv": warr(W_qkv[:, 2 * C:][:, cols]),
            "wp": _to_mm_dtype(wp2),
        })
    return in_maps


_CACHE = {}
TRACE = False           # set True (e.g. from test.py) to capture an NTFF profile


def kernel(X, W_qkv, W_proj):
    import sys
    if "/opt/trn_rl_repo" not in sys.path:
        sys.path.insert(0, "/opt/trn_rl_repo")
    from concourse.bass_utils import run_bass_kernel_spmd

    X = np.asarray(X, dtype=np.float32)
    W_qkv = np.asarray(W_qkv, dtype=np.float32)
    W_proj = np.asarray(W_proj, dtype=np.float32)

    if "nc" not in _CACHE:
        _CACHE["nc"] = build_nc()
    nc = _CACHE["nc"]

    in_maps = make_in_maps(X, W_qkv, W_proj)
    res = run_bass_kernel_spmd(nc, in_maps, core_ids=list(range(NCORES)),
                               trace=TRACE)
    _CACHE["last"] = res
    out = np.empty((B, T, C), dtype=np.float32)
    ncb = NCORES // B
    for b in range(B):
        acc = res.results[b * ncb]["out"].astype(np.float32)
        for c in range(b * ncb + 1, (b + 1) * ncb):
            acc = acc + res.results[c]["out"].astype(np.float32)
        out[b] = acc
    return out


# revision 39
# speedup vs baseline: 1.1412x; 1.0036x over previous
"""Causal self-attention Trainium2 kernel (Bass/Tile), 8-core SPMD.

Problem: X[2, 2048, 1024], W_qkv[1024, 3072], W_proj[1024, 1024], H=16 heads.

Sharding: core c handles batch b = c // 4 and heads h0 = 4*(c % 4) .. h0+4
(tensor-parallel over heads + data-parallel over batch). Each core computes
a partial output  out_b = Y[:, heads] @ W_proj[head rows, :]  and the host
sums the 4 partials per batch (the W_proj row-shard reduction).

Per-core device layout ("transposed attention", no P transposes needed):
  Xt  [C, T]      X[b].T, all 32 [128, 512] tiles DMAed up front
  Qt,Kt [128,2,CH] per chunk, per head-pair group g: partition = 64*(h%2)+d
  V   [128,4,260] per chunk [token-block, head*65(+ones col)] for PV lhsT
  St  = Kt_blk.T @ Qt_chunk -> [keys 128, q 512] PSUM (K=d=64 contraction),
        both heads of a pair packed side-by-side in one [128, 1024] tile
  P   = exp(0.125*(St + causal_mask))  via ACT; ones-augmented PV gives
  Yt_aug = [V|1].T @ P -> [65, q 512]: rows 0-63 = Yt, row 64 = softmax sums

Scheduling: the TRN2 PE clock ramps 0.65 -> 1.2 -> 2.4 GHz with ~3us of
CONTINUOUS execution and drops back on idle, so the whole kernel is emitted
as one dense PE stream: warmup matmuls cover the initial weight DMAs, and
the QKV projection for token chunk tch+1 plus the output projection for
chunk qi-1 are injected one group per key-block iteration into the
attention loop over chunk qi, so the PE never starves while ACT runs exp.
ST matmuls and exp skip the fully-masked columns left of the causal
diagonal; PV reads only the surviving columns.
"""

import numpy as np
from collections import deque

B, T, C, H = 2, 2048, 1024, 16
HD = 64          # head dim
HPC = 4          # heads per core
P = 128
NCORES = 8
CH = 512         # token chunk (matmul free dim / q chunk)
KB = 128         # key block
MASK_VAL = -1.0e5
MM_DTYPE = "bf16"


def build_nc(t_len=T, mm_dtype=None):
    import concourse.bass as bass
    import concourse.mybir as mybir
    from concourse import bacc, library_config
    from concourse.tile import TileContext
    from contextlib import ExitStack

    mm_dtype = mm_dtype or MM_DTYPE
    f32 = mybir.dt.float32
    mdt = mybir.dt.bfloat16 if mm_dtype == "bf16" else mybir.dt.float32r
    Exp = mybir.ActivationFunctionType.Exp
    Alu = mybir.AluOpType

    NKC = C // P          # 8 contraction chunks over C
    NCH = t_len // CH     # token chunks
    NBC = CH // P         # token blocks per chunk (4)
    SC = 1.0 / np.sqrt(HD)

    nc = bacc.Bacc("TRN2", target_bir_lowering=False, debug=False,
                   num_devices=NCORES)

    xt_d = nc.dram_tensor("xt", [C, t_len], mdt, kind="ExternalInput").ap()
    wq_d = nc.dram_tensor("wq", [P, NKC, HPC * HD], mdt, kind="ExternalInput").ap()
    wk_d = nc.dram_tensor("wk", [P, NKC, HPC * HD], mdt, kind="ExternalInput").ap()
    wv_d = nc.dram_tensor("wv", [P, NKC, HPC * HD], mdt, kind="ExternalInput").ap()
    wp_d = nc.dram_tensor("wp", [P, 2, C], mdt, kind="ExternalInput").ap()
    out_d = nc.dram_tensor("out", [t_len, C], mdt, kind="ExternalOutput").ap()

    with TileContext(nc) as tc, ExitStack() as ctx:
        const = ctx.enter_context(tc.tile_pool(name="const", bufs=1))
        work = ctx.enter_context(tc.tile_pool(name="work", bufs=3))
        psm = ctx.enter_context(tc.tile_pool(name="psm", bufs=2, space="PSUM"))

        # ---- persistent SBUF tensors ----
        wq_sb = const.tile([P, NKC, HPC * HD], mdt, tag="wq")
        wk_sb = const.tile([P, NKC, HPC * HD], mdt, tag="wk")
        wv_sb = const.tile([P, NKC, HPC * HD], mdt, tag="wv")
        wp_sb = const.tile([P, 2, C], mdt, tag="wp")
        # per-chunk tensors (separate tiles -> no false cross-chunk deps)
        qt = [const.tile([P, 2, CH], mdt, tag=f"qt{t}", name=f"qt{t}")
              for t in range(NCH)]
        kt = [const.tile([P, 2, CH], mdt, tag=f"kt{t}", name=f"kt{t}")
              for t in range(NCH)]
        va = [const.tile([P, NBC, HPC * (HD + 1)], mdt, tag=f"va{t}",
                         name=f"va{t}") for t in range(NCH)]
        y2 = [const.tile([P, 2, CH], mdt, tag=f"y2{t}", name=f"y2{t}")
              for t in range(NCH)]
        xts = [[const.tile([P, CH], mdt, tag=f"xt{t}_{kc}", name=f"xt{t}_{kc}")
                for kc in range(NKC)] for t in range(NCH)]

        def xsrc(tch, kc, c0=0, c1=CH):
            return xts[tch][kc][:, c0:c1]
        warm = const.tile([P, CH], mdt, tag="warm")

        # ---- PE warmup: dense junk matmuls so the tensor engine p-state
        # ramps while the first weight/activation DMAs are in flight.  The
        # memset is the FIRST emitted instruction so nothing can precede it
        # in the DVE queue and stall the PE stream behind it. ----
        nc.vector.memset(warm[:], 0.0)
        wt = psm.tile([P, 2 * CH], f32, tag="st", bufs=3, name="warm")[:, :CH]
        NW = 16
        for i in range(NW):
            nc.tensor.matmul(wt[:], lhsT=warm[:, :P], rhs=warm[:],
                             start=(i == 0), stop=(i == NW - 1))
        nc.gpsimd.load_library(library_config.proxy)

        # ---- input DMAs: wq + chunk-0 Xt first so QKV(0) can start; wk/wv
        # issued early (Sync issues DMAs serially at ~0.6us each and the
        # k/v matmul groups need them before the Xt tail) ----
        nc.sync.dma_start(wq_sb[:, :NKC // 2], wq_d[:, :NKC // 2])
        nc.sync.dma_start(xts[0][0][:], xt_d[0:P, 0:CH])
        nc.sync.dma_start(wq_sb[:, NKC // 2:], wq_d[:, NKC // 2:])
        nc.sync.dma_start(xts[0][1][:], xt_d[P:2 * P, 0:CH])
        nc.sync.dma_start(wk_sb[:, :NKC // 2], wk_d[:, :NKC // 2])
        nc.sync.dma_start(wk_sb[:, NKC // 2:], wk_d[:, NKC // 2:])
        for kc in range(2, NKC):
            nc.sync.dma_start(xts[0][kc][:], xt_d[kc * P:(kc + 1) * P, 0:CH])
        nc.sync.dma_start(wv_sb[:, :NKC // 2], wv_d[:, :NKC // 2])
        nc.sync.dma_start(wv_sb[:, NKC // 2:], wv_d[:, NKC // 2:])
        nc.sync.dma_start(wp_sb[:], wp_d[:])
        for tch in range(1, NCH):
            for kc in range(NKC):
                nc.sync.dma_start(
                    xts[tch][kc][:],
                    xt_d[kc * P:(kc + 1) * P, tch * CH:(tch + 1) * CH])

        # ones columns of the augmented V (softmax denominator trick).  The
        # ones column comes FIRST in each head's 65-block so the PV sums row
        # lands in PSUM partition 0, where partition_broadcast can read it
        # without a bounce DMA.
        for t in range(NCH):
            for hl in range(HPC):
                nc.vector.memset(
                    va[t][:, :, hl * (HD + 1): hl * (HD + 1) + 1], 1.0)

        # ---- QKV projection for one token chunk: 8 injectable PE groups ----
        def qkv_group_thunks(tch):
            thunks = []

            def qk_group(w_sb, dst, g):
                def run():
                    pq = psm.tile([P, 2 * CH], f32, tag="st", bufs=3,
                                  name="pq")[:, :CH]
                    for kc in range(NKC):
                        nc.tensor.matmul(
                            pq[:],
                            lhsT=w_sb[:, kc, g * P:(g + 1) * P],
                            rhs=xsrc(tch, kc),
                            start=(kc == 0), stop=(kc == NKC - 1))
                    nc.vector.tensor_copy(out=dst[tch][:, g, :], in_=pq[:])
                return run

            def v_group(vb):
                def run():
                    pv = psm.tile([P, 2 * CH], f32, tag="st", bufs=3,
                                  name="pv")[:, :HPC * HD]
                    for kc in range(NKC):
                        nc.tensor.matmul(
                            pv,
                            lhsT=xsrc(tch, kc, vb * P, (vb + 1) * P),
                            rhs=wv_sb[:, kc, :],
                            start=(kc == 0), stop=(kc == NKC - 1))
                    nc.vector.tensor_copy(
                        out=va[tch][:, vb, :]
                        .rearrange("p (h e) -> p h e", e=HD + 1)[:, :, 1:],
                        in_=pv.rearrange("p (h e) -> p h e", e=HD))
                return run

            for g in range(2):
                thunks.append(qk_group(wq_sb, qt, g))
                thunks.append(qk_group(wk_sb, kt, g))
            for vb in range(NBC):
                thunks.append(v_group(vb))
            return thunks

        # ---- output projection for one chunk: 8 injectable PE pairs ----
        def proj_pair_thunks(qi):
            thunks = []
            last = qi == NCH - 1

            def pair(tbl, oc):
                def run():
                    pp = psm.tile([P, 2 * CH], f32, tag="st", bufs=3,
                                  name="pp")[:, :CH]
                    for yc in range(2):
                        nc.tensor.matmul(
                            pp[:],
                            lhsT=y2[qi][:, yc, tbl * P:(tbl + 1) * P],
                            rhs=wp_sb[:, yc, oc * CH:(oc + 1) * CH],
                            start=(yc == 0), stop=(yc == 1))
                    ost = work.tile([P, CH], mdt, tag="ost", bufs=3)
                    # on the final chunk ACT is idle: alternate the PSUM
                    # drains between ACT and DVE so the pp slots free at
                    # twice the rate and the PE never waits on a drain
                    if last and oc == 0:
                        nc.scalar.copy(out=ost[:], in_=pp[:])
                    else:
                        nc.vector.tensor_copy(out=ost[:], in_=pp[:])
                    tb = qi * NBC + tbl
                    nc.sync.dma_start(
                        out_d[tb * P:(tb + 1) * P, oc * CH:(oc + 1) * CH],
                        ost[:])
                return run

            for tbl in range(NBC):
                for oc in range(2):
                    thunks.append(pair(tbl, oc))
            return thunks

        # ---- normalize Yt rows by the sums rows (ytp partition 0), store
        # into y2 via partition-shifting SBUF->SBUF DMAs.  Both heads of the
        # pair share one broadcast/reciprocal/multiply over [65, 2*CH]. ----
        def normalize_pair(g, qi, ytps, last=False):
            """The PSUM tiles are drained immediately (copies) so the pool
            slots free before the reciprocal chain completes.  On the final
            chunk the hh=1 copy moves to ACT (idle by then) so the two
            drains don't serialize on DVE ahead of the last proj."""
            bnc = work.tile([HD + 1, 2, CH], f32, tag="bounce", bufs=2)
            nc.vector.tensor_copy(out=bnc[:, 0, :], in_=ytps[0][:])
            if last:
                nc.scalar.copy(out=bnc[:, 1, :], in_=ytps[1][:])
            else:
                nc.vector.tensor_copy(out=bnc[:, 1, :], in_=ytps[1][:])
            # partition_broadcast / custom-DVE ops ignore the AP base
            # partition on HW; the sums rows are at partition 0 by layout.
            rb = work.tile([HD + 1, 2, CH], f32, tag="rb", bufs=2)
            nc.gpsimd.partition_broadcast(rb[:], bnc[0:1, :, :])
            nc.vector.reciprocal_approx_fast(out=rb[:], in_=rb[:])
            ybs = work.tile([HD + 1, 2, CH], mdt, tag="ybs", bufs=2)
            # row 0 computes sums * 1/sums; only rows 1..64 are stored.
            # (DVE partition base must be aligned, so operate on [0:65].)
            nc.vector.tensor_tensor(out=ybs[:], in0=bnc[:], in1=rb[:],
                                    op=Alu.mult)
            for hh in range(2):
                # shift each head's 64 rows to partitions hh*64 .. hh*64+63
                nc.sync.dma_start(y2[qi][hh * HD:(hh + 1) * HD, g, :],
                                  ybs[1:, hh, :])

        # ---- attention for one query chunk, with PE work injection ----
        def attention(qi, queue, start_at=0):
            nkb = NBC * (qi + 1)
            it = 0
            for g in range(2):
                ytps = [psm.tile([HD + 1, CH], f32, tag="yt", bufs=2,
                                 name=f"ytp{hh}") for hh in range(2)]

                def emit_pv(pts, kb, qoff):
                    for hh in range(2):
                        hl = 2 * g + hh
                        nc.tensor.matmul(
                            ytps[hh][:, qoff:],
                            lhsT=va[kb // NBC][:, kb % NBC,
                                               hl * (HD + 1):(hl + 1) * (HD + 1)],
                            rhs=pts[:, hh * CH + qoff:(hh + 1) * CH],
                            start=(kb == 0), stop=(kb == nkb - 1))

                prev = None
                for kb in range(nkb):
                    o4 = kb - NBC * qi          # 0..3 on the diagonal band
                    qoff = max(0, o4) * KB
                    stp = psm.tile([P, 2 * CH], f32, tag="st", bufs=3,
                                   name="stp")
                    for hh in range(2):
                        nc.tensor.matmul(
                            stp[:, hh * CH + qoff:(hh + 1) * CH],
                            lhsT=kt[kb // NBC][hh * HD:(hh + 1) * HD, g,
                                               (kb % NBC) * KB:(kb % NBC + 1) * KB],
                            rhs=qt[qi][hh * HD:(hh + 1) * HD, g, qoff:],
                            start=True, stop=True)
                    # PV of the previous block slots in while this block's
                    # exp runs, keeping PE fed (software pipelining).
                    if prev is not None:
                        emit_pv(*prev)
                    if queue and it >= start_at:
                        n = -(-len(queue) // max(1, 2 * nkb - it))
                        for _ in range(min(n, len(queue))):
                            queue.popleft()()
                    it += 1
                    pts = work.tile([P, 2 * CH], mdt, tag="p", bufs=4,
                                    name="pts")
                    if o4 >= 0:
                        nc.scalar.activation(
                            out=pts.rearrange("p (h c) -> p h c", h=2)[:, :, qoff:],
                            in_=stp.rearrange("p (h c) -> p h c", h=2)[:, :, qoff:],
                            func=Exp, scale=SC)
                        # causal mask: zero the upper triangle of the
                        # 128-wide diagonal band of P (keep where q >= k,
                        # i.e. band column f >= partition p).  Runs on
                        # GpSimd so the DVE queue never delays exp or PV.
                        for hh in range(2):
                            nc.gpsimd.affine_select(
                                out=pts[:, hh * CH + qoff:hh * CH + qoff + KB],
                                in_=pts[:, hh * CH + qoff:hh * CH + qoff + KB],
                                compare_op=Alu.is_ge, fill=0.0,
                                base=0, channel_multiplier=-1,
                                pattern=[[1, KB]])
                    else:
                        nc.scalar.activation(out=pts[:], in_=stp[:],
                                             func=Exp, scale=SC)
                    prev = (pts, kb, qoff)
                emit_pv(*prev)
                normalize_pair(g, qi, ytps,
                               last=(qi == NCH - 1 and g == 1))

        # ---- emission schedule ----
        def junk(n):
            """Dependency-free matmuls bridging a known PE stall so the
            p-state stays at max while DMAs land."""
            jt = psm.tile([P, 2 * CH], f32, tag="st", bufs=3,
                          name="junk")[:, :CH]
            for i in range(n):
                nc.tensor.matmul(jt[:], lhsT=warm[:, :P], rhs=warm[:],
                                 start=(i == 0), stop=(i == n - 1))

        # phase A: only q-g0/k-g0 of chunk 0 up front; the v-groups and the
        # g1 projections are injected into attention(0) just ahead of need.
        # The initial weight/Xt DMAs land slower than the PE consumes, so
        # junk bridges keep the clock hot between the first groups.
        t0 = qkv_group_thunks(0)
        t0[0]()
        junk(5)
        t0[1]()
        junk(5)
        for qi in range(NCH):
            queue = deque()
            if qi == 0:
                queue.extend([t0[4], t0[5], t0[2], t0[3], t0[6], t0[7]])
            if qi + 1 < NCH:
                queue.extend(qkv_group_thunks(qi + 1))
            start_at = 0
            if qi > 0:
                queue.extend(proj_pair_thunks(qi - 1))
                start_at = 3 if qi == NCH - 1 else 0
            attention(qi, queue, start_at)
            while queue:
                queue.popleft()()
        # bridge: junk matmuls keep the PE p-state at max while the last
        # chunk's normalize chains run, so the final proj starts hot and
        # immediately.
        wt2 = psm.tile([P, 2 * CH], f32, tag="st", bufs=3,
                       name="bridge")[:, :CH]
        NB = 34
        for i in range(NB):
            nc.tensor.matmul(wt2[:], lhsT=warm[:, :P], rhs=warm[:],
                             start=(i == 0), stop=(i == NB - 1))
        for th in proj_pair_thunks(NCH - 1):
            th()
    nc.compile()
    return nc


def _to_mm_dtype(a):
    if MM_DTYPE == "bf16":
        import ml_dtypes
        return np.ascontiguousarray(a).astype(ml_dtypes.bfloat16)
    return np.ascontiguousarray(a).astype(np.float32)


def make_in_maps(X, W_qkv, W_proj, t_len=T):
    """Host-side sharding: slice + pre-arrange weights per head group,
    transpose X.  Layouts match the SBUF tensors so every weight DMA is
    fully contiguous:
      wq/wk/wv [128, 8, 256]: [p, kc, m] = W[kc*128+p, cols][m]
      wp       [128, 2, C]:   [64*hh+d, yc, m] = W_proj[(2*yc+hh)*64+d, m]
    """
    in_maps = []
    xts = [_to_mm_dtype(np.asarray(X[b, :t_len, :]).T) for b in range(B)]
    NKC = C // P
    for c in range(NCORES):
        b = c // (NCORES // B)
        h0 = HPC * (c % (NCORES // B))
        cols = slice(h0 * HD, (h0 + HPC) * HD)

        def warr(w):
            return _to_mm_dtype(
                np.ascontiguousarray(w).reshape(NKC, P, HPC * HD)
                .transpose(1, 0, 2))

        wp_c = np.ascontiguousarray(W_proj[cols, :])          # [256, C]
        wp2 = wp_c.reshape(2, 2, HD, C).transpose(1, 2, 0, 3).reshape(P, 2, C)
        in_maps.append({
            "xt": xts[b],
            "wq": warr(W_qkv[:, cols]),
            "wk": warr(W_qkv[:, C:][:, cols]),
            "wv": warr(W_qkv[:, 2 * C:][:, cols]),
            "wp": _to_mm_dtype(wp2),
        })
    return in_maps


_CACHE = {}
TRACE = False           # set True (e.g. from test.py) to capture an NTFF profile


def kernel(X, W_qkv, W_proj):
    import sys
    if "/opt/trn_rl_repo" not in sys.path:
        sys.path.insert(0, "/opt/trn_rl_repo")
    from concourse.bass_utils import run_bass_kernel_spmd

    X = np.asarray(X, dtype=np.float32)
    W_qkv = np.asarray(W_qkv, dtype=np.float32)
    W_proj = np.asarray(W_proj, dtype=np.float32)

    if "nc" not in _CACHE:
        _CACHE["nc"] = build_nc()
    nc = _CACHE["nc"]

    in_maps = make_in_maps(X, W_qkv, W_proj)
    res = run_bass_kernel_spmd(nc, in_maps, core_ids=list(range(NCORES)),
                               trace=TRACE)
    _CACHE["last"] = res
    out = np.empty((B, T, C), dtype=np.float32)
    ncb = NCORES // B
    for b in range(B):
        acc = res.results[b * ncb]["out"].astype(np.float32)
        for c in range(b * ncb + 1, (b + 1) * ncb):
            acc = acc + res.results[c]["out"].astype(np.float32)
        out[b] = acc
    return out


# revision 40
# speedup vs baseline: 1.1657x; 1.0215x over previous
"""Causal self-attention Trainium2 kernel (Bass/Tile), 8-core SPMD.

Problem: X[2, 2048, 1024], W_qkv[1024, 3072], W_proj[1024, 1024], H=16 heads.

Sharding: core c handles batch b = c // 4 and heads h0 = 4*(c % 4) .. h0+4
(tensor-parallel over heads + data-parallel over batch). Each core computes
a partial output  out_b = Y[:, heads] @ W_proj[head rows, :]  and the host
sums the 4 partials per batch (the W_proj row-shard reduction).

Per-core device layout ("transposed attention", no P transposes needed):
  Xt  [C, T]      X[b].T, all 32 [128, 512] tiles DMAed up front
  Qt,Kt [128,2,CH] per chunk, per head-pair group g: partition = 64*(h%2)+d
  V   [128,4,260] per chunk [token-block, head*65(+ones col)] for PV lhsT
  St  = Kt_blk.T @ Qt_chunk -> [keys 128, q 512] PSUM (K=d=64 contraction),
        both heads of a pair packed side-by-side in one [128, 1024] tile
  P   = exp(0.125*(St + causal_mask))  via ACT; ones-augmented PV gives
  Yt_aug = [V|1].T @ P -> [65, q 512]: rows 0-63 = Yt, row 64 = softmax sums

Scheduling: the TRN2 PE clock ramps 0.65 -> 1.2 -> 2.4 GHz with ~3us of
CONTINUOUS execution and drops back on idle, so the whole kernel is emitted
as one dense PE stream: warmup matmuls cover the initial weight DMAs, and
the QKV projection for token chunk tch+1 plus the output projection for
chunk qi-1 are injected one group per key-block iteration into the
attention loop over chunk qi, so the PE never starves while ACT runs exp.
ST matmuls and exp skip the fully-masked columns left of the causal
diagonal; PV reads only the surviving columns.
"""

import numpy as np
from collections import deque

B, T, C, H = 2, 2048, 1024, 16
HD = 64          # head dim
HPC = 4          # heads per core
P = 128
NCORES = 8
CH = 512         # token chunk (matmul free dim / q chunk)
KB = 128         # key block
MASK_VAL = -1.0e5
MM_DTYPE = "bf16"


def build_nc(t_len=T, mm_dtype=None):
    import concourse.bass as bass
    import concourse.mybir as mybir
    from concourse import bacc, library_config
    from concourse.tile import TileContext
    from contextlib import ExitStack

    mm_dtype = mm_dtype or MM_DTYPE
    f32 = mybir.dt.float32
    mdt = mybir.dt.bfloat16 if mm_dtype == "bf16" else mybir.dt.float32r
    Exp = mybir.ActivationFunctionType.Exp
    Alu = mybir.AluOpType

    NKC = C // P          # 8 contraction chunks over C
    NCH = t_len // CH     # token chunks
    NBC = CH // P         # token blocks per chunk (4)
    SC = 1.0 / np.sqrt(HD)

    nc = bacc.Bacc("TRN2", target_bir_lowering=False, debug=False,
                   num_devices=NCORES)

    xt_d = nc.dram_tensor("xt", [C, t_len], mdt, kind="ExternalInput").ap()
    wq_d = nc.dram_tensor("wq", [P, NKC, HPC * HD], mdt, kind="ExternalInput").ap()
    wk_d = nc.dram_tensor("wk", [P, NKC, HPC * HD], mdt, kind="ExternalInput").ap()
    wv_d = nc.dram_tensor("wv", [P, NKC, HPC * HD], mdt, kind="ExternalInput").ap()
    wp_d = nc.dram_tensor("wp", [P, 2, C], mdt, kind="ExternalInput").ap()
    out_d = nc.dram_tensor("out", [t_len, C], mdt, kind="ExternalOutput").ap()

    with TileContext(nc) as tc, ExitStack() as ctx:
        const = ctx.enter_context(tc.tile_pool(name="const", bufs=1))
        work = ctx.enter_context(tc.tile_pool(name="work", bufs=3))
        psm = ctx.enter_context(tc.tile_pool(name="psm", bufs=2, space="PSUM"))

        # ---- persistent SBUF tensors ----
        wq_sb = const.tile([P, NKC, HPC * HD], mdt, tag="wq")
        wk_sb = const.tile([P, NKC, HPC * HD], mdt, tag="wk")
        wv_sb = const.tile([P, NKC, HPC * HD], mdt, tag="wv")
        wp_sb = const.tile([P, 2, C], mdt, tag="wp")
        # per-chunk tensors (separate tiles -> no false cross-chunk deps)
        qt = [const.tile([P, 2, CH], mdt, tag=f"qt{t}", name=f"qt{t}")
              for t in range(NCH)]
        kt = [const.tile([P, 2, CH], mdt, tag=f"kt{t}", name=f"kt{t}")
              for t in range(NCH)]
        VE = HD + 8     # padded per-head V-block stride (ones, 64 dims, pad)
        va = [const.tile([P, NBC, HPC * VE], mdt, tag=f"va{t}",
                         name=f"va{t}") for t in range(NCH)]
        y2 = [const.tile([P, 2, CH], mdt, tag=f"y2{t}", name=f"y2{t}")
              for t in range(NCH)]
        xts = [[const.tile([P, CH], mdt, tag=f"xt{t}_{kc}", name=f"xt{t}_{kc}")
                for kc in range(NKC)] for t in range(NCH)]

        def xsrc(tch, kc, c0=0, c1=CH):
            return xts[tch][kc][:, c0:c1]
        warm = const.tile([P, CH], mdt, tag="warm")

        # ---- PE warmup: dense junk matmuls so the tensor engine p-state
        # ramps while the first weight/activation DMAs are in flight.  The
        # memset is the FIRST emitted instruction so nothing can precede it
        # in the DVE queue and stall the PE stream behind it. ----
        nc.vector.memset(warm[:], 0.0)
        wt = psm.tile([P, 2 * CH], f32, tag="st", bufs=3, name="warm")[:, :CH]
        NW = 16
        for i in range(NW):
            nc.tensor.matmul(wt[:], lhsT=warm[:, :P], rhs=warm[:],
                             start=(i == 0), stop=(i == NW - 1))
        nc.gpsimd.load_library(library_config.proxy)

        # ---- input DMAs: wq + chunk-0 Xt first so QKV(0) can start; wk/wv
        # issued early (Sync issues DMAs serially at ~0.6us each and the
        # k/v matmul groups need them before the Xt tail) ----
        nc.sync.dma_start(wq_sb[:, :NKC // 2], wq_d[:, :NKC // 2])
        nc.sync.dma_start(xts[0][0][:], xt_d[0:P, 0:CH])
        nc.sync.dma_start(wq_sb[:, NKC // 2:], wq_d[:, NKC // 2:])
        nc.sync.dma_start(xts[0][1][:], xt_d[P:2 * P, 0:CH])
        nc.sync.dma_start(wk_sb[:, :NKC // 2], wk_d[:, :NKC // 2])
        nc.sync.dma_start(wk_sb[:, NKC // 2:], wk_d[:, NKC // 2:])
        for kc in range(2, NKC):
            nc.sync.dma_start(xts[0][kc][:], xt_d[kc * P:(kc + 1) * P, 0:CH])
        nc.sync.dma_start(wv_sb[:, :NKC // 2], wv_d[:, :NKC // 2])
        nc.sync.dma_start(wv_sb[:, NKC // 2:], wv_d[:, NKC // 2:])
        nc.sync.dma_start(wp_sb[:], wp_d[:])
        for tch in range(1, NCH):
            for kc in range(NKC):
                nc.sync.dma_start(
                    xts[tch][kc][:],
                    xt_d[kc * P:(kc + 1) * P, tch * CH:(tch + 1) * CH])

        # ones columns of the augmented V (softmax denominator trick).  The
        # ones column comes FIRST in each head's 65-block so the PV sums row
        # lands in PSUM partition 0, where partition_broadcast can read it
        # without a bounce DMA.
        for t in range(NCH):
            for hl in range(HPC):
                nc.vector.memset(
                    va[t][:, :, hl * VE: hl * VE + 1], 1.0)

        # ---- QKV projection for one token chunk: 8 injectable PE groups ----
        def qkv_group_thunks(tch):
            thunks = []

            def qk_group(w_sb, dst, g):
                def run():
                    pq = psm.tile([P, 2 * CH], f32, tag="st", bufs=3,
                                  name="pq")[:, :CH]
                    for kc in range(NKC):
                        nc.tensor.matmul(
                            pq[:],
                            lhsT=w_sb[:, kc, g * P:(g + 1) * P],
                            rhs=xsrc(tch, kc),
                            start=(kc == 0), stop=(kc == NKC - 1))
                    nc.vector.tensor_copy(out=dst[tch][:, g, :], in_=pq[:])
                return run

            def v_group(vb):
                def run():
                    pv = psm.tile([P, 2 * CH], f32, tag="st", bufs=3,
                                  name="pv")[:, :HPC * HD]
                    for kc in range(NKC):
                        nc.tensor.matmul(
                            pv,
                            lhsT=xsrc(tch, kc, vb * P, (vb + 1) * P),
                            rhs=wv_sb[:, kc, :],
                            start=(kc == 0), stop=(kc == NKC - 1))
                    nc.vector.tensor_copy(
                        out=va[tch][:, vb, :]
                        .rearrange("p (h e) -> p h e", e=VE)[:, :, 1:HD + 1],
                        in_=pv.rearrange("p (h e) -> p h e", e=HD))
                return run

            for g in range(2):
                thunks.append(qk_group(wq_sb, qt, g))
                thunks.append(qk_group(wk_sb, kt, g))
            for vb in range(NBC):
                thunks.append(v_group(vb))
            return thunks

        # ---- output projection for one chunk: 8 injectable PE pairs ----
        def proj_pair_thunks(qi):
            thunks = []
            last = qi == NCH - 1

            def pair(tbl, oc):
                def run():
                    pp = psm.tile([P, 2 * CH], f32, tag="st", bufs=3,
                                  name="pp")[:, :CH]
                    for yc in range(2):
                        nc.tensor.matmul(
                            pp[:],
                            lhsT=y2[qi][:, yc, tbl * P:(tbl + 1) * P],
                            rhs=wp_sb[:, yc, oc * CH:(oc + 1) * CH],
                            start=(yc == 0), stop=(yc == 1))
                    ost = work.tile([P, CH], mdt, tag="ost", bufs=3)
                    # on the final chunk ACT is idle: alternate the PSUM
                    # drains between ACT and DVE so the pp slots free at
                    # twice the rate and the PE never waits on a drain
                    if last and oc == 0:
                        nc.scalar.copy(out=ost[:], in_=pp[:])
                    else:
                        nc.vector.tensor_copy(out=ost[:], in_=pp[:])
                    tb = qi * NBC + tbl
                    nc.sync.dma_start(
                        out_d[tb * P:(tb + 1) * P, oc * CH:(oc + 1) * CH],
                        ost[:])
                return run

            for tbl in range(NBC):
                for oc in range(2):
                    thunks.append(pair(tbl, oc))
            return thunks

        # ---- normalize Yt rows by the sums rows (ytp partition 0), store
        # into y2 via partition-shifting SBUF->SBUF DMAs.  Both heads of the
        # pair share one broadcast/reciprocal/multiply over [65, 2*CH]. ----
        def normalize_pair(g, qi, ytps, last=False):
            """The PSUM tiles are drained immediately (copies) so the pool
            slots free before the reciprocal chain completes.  On the final
            chunk the hh=1 copy moves to ACT (idle by then) so the two
            drains don't serialize on DVE ahead of the last proj."""
            bnc = work.tile([HD + 1, 2, CH], f32, tag="bounce", bufs=2)
            nc.vector.tensor_copy(out=bnc[:, 0, :], in_=ytps[0][:])
            if last:
                nc.scalar.copy(out=bnc[:, 1, :], in_=ytps[1][:])
            else:
                nc.vector.tensor_copy(out=bnc[:, 1, :], in_=ytps[1][:])
            # partition_broadcast / custom-DVE ops ignore the AP base
            # partition on HW; the sums rows are at partition 0 by layout.
            rb = work.tile([HD + 1, 2, CH], f32, tag="rb", bufs=2)
            nc.gpsimd.partition_broadcast(rb[:], bnc[0:1, :, :])
            nc.vector.reciprocal_approx_fast(out=rb[:], in_=rb[:])
            ybs = work.tile([HD + 1, 2, CH], mdt, tag="ybs", bufs=2)
            # row 0 computes sums * 1/sums; only rows 1..64 are stored.
            # (DVE partition base must be aligned, so operate on [0:65].)
            nc.vector.tensor_tensor(out=ybs[:], in0=bnc[:], in1=rb[:],
                                    op=Alu.mult)
            for hh in range(2):
                # shift each head's 64 rows to partitions hh*64 .. hh*64+63
                nc.sync.dma_start(y2[qi][hh * HD:(hh + 1) * HD, g, :],
                                  ybs[1:, hh, :])

        # ---- attention for one query chunk, with PE work injection ----
        def attention(qi, queue, start_at=0):
            nkb = NBC * (qi + 1)
            it = 0
            for g in range(2):
                ytps = [psm.tile([HD + 1, CH], f32, tag="yt", bufs=2,
                                 name=f"ytp{hh}") for hh in range(2)]

                def emit_pv(pts, kb, qoff):
                    for hh in range(2):
                        hl = 2 * g + hh
                        nc.tensor.matmul(
                            ytps[hh][:, qoff:],
                            lhsT=va[kb // NBC][:, kb % NBC,
                                               hl * VE:hl * VE + HD + 1],
                            rhs=pts[:, hh * CH + qoff:(hh + 1) * CH],
                            start=(kb == 0), stop=(kb == nkb - 1))

                prev = None
                for kb in range(nkb):
                    o4 = kb - NBC * qi          # 0..3 on the diagonal band
                    qoff = max(0, o4) * KB
                    stp = psm.tile([P, 2 * CH], f32, tag="st", bufs=3,
                                   name="stp")
                    for hh in range(2):
                        nc.tensor.matmul(
                            stp[:, hh * CH + qoff:(hh + 1) * CH],
                            lhsT=kt[kb // NBC][hh * HD:(hh + 1) * HD, g,
                                               (kb % NBC) * KB:(kb % NBC + 1) * KB],
                            rhs=qt[qi][hh * HD:(hh + 1) * HD, g, qoff:],
                            start=True, stop=True)
                    # PV of the previous block slots in while this block's
                    # exp runs, keeping PE fed (software pipelining).
                    if prev is not None:
                        emit_pv(*prev)
                    if queue and it >= start_at:
                        n = -(-len(queue) // max(1, 2 * nkb - it))
                        for _ in range(min(n, len(queue))):
                            queue.popleft()()
                    it += 1
                    pts = work.tile([P, 2 * CH], mdt, tag="p", bufs=4,
                                    name="pts")
                    if o4 >= 0:
                        nc.scalar.activation(
                            out=pts.rearrange("p (h c) -> p h c", h=2)[:, :, qoff:],
                            in_=stp.rearrange("p (h c) -> p h c", h=2)[:, :, qoff:],
                            func=Exp, scale=SC)
                        # causal mask: zero the upper triangle of the
                        # 128-wide diagonal band of P (keep where q >= k,
                        # i.e. band column f >= partition p).  Runs on
                        # GpSimd so the DVE queue never delays exp or PV.
                        for hh in range(2):
                            nc.gpsimd.affine_select(
                                out=pts[:, hh * CH + qoff:hh * CH + qoff + KB],
                                in_=pts[:, hh * CH + qoff:hh * CH + qoff + KB],
                                compare_op=Alu.is_ge, fill=0.0,
                                base=0, channel_multiplier=-1,
                                pattern=[[1, KB]])
                    else:
                        nc.scalar.activation(out=pts[:], in_=stp[:],
                                             func=Exp, scale=SC)
                    prev = (pts, kb, qoff)
                emit_pv(*prev)
                normalize_pair(g, qi, ytps,
                               last=(qi == NCH - 1 and g == 1))

        # ---- emission schedule ----
        def junk(n):
            """Dependency-free matmuls bridging a known PE stall so the
            p-state stays at max while DMAs land."""
            jt = psm.tile([P, 2 * CH], f32, tag="st", bufs=3,
                          name="junk")[:, :CH]
            for i in range(n):
                nc.tensor.matmul(jt[:], lhsT=warm[:, :P], rhs=warm[:],
                                 start=(i == 0), stop=(i == n - 1))

        # phase A: only q-g0/k-g0 of chunk 0 up front; the v-groups and the
        # g1 projections are injected into attention(0) just ahead of need.
        # The initial weight/Xt DMAs land slower than the PE consumes, so
        # junk bridges keep the clock hot between the first groups.
        t0 = qkv_group_thunks(0)
        t0[0]()
        junk(5)
        t0[1]()
        junk(5)
        for qi in range(NCH):
            queue = deque()
            if qi == 0:
                queue.extend([t0[4], t0[5], t0[2], t0[3], t0[6], t0[7]])
            if qi + 1 < NCH:
                queue.extend(qkv_group_thunks(qi + 1))
            start_at = 0
            if qi > 0:
                queue.extend(proj_pair_thunks(qi - 1))
                start_at = 5 if qi == NCH - 1 else 0
            attention(qi, queue, start_at)
            while queue:
                queue.popleft()()
        # bridge: junk matmuls keep the PE p-state at max while the last
        # chunk's normalize chains run, so the final proj starts hot and
        # immediately.
        wt2 = psm.tile([P, 2 * CH], f32, tag="st", bufs=3,
                       name="bridge")[:, :CH]
        NB = 34
        for i in range(NB):
            nc.tensor.matmul(wt2[:], lhsT=warm[:, :P], rhs=warm[:],
                             start=(i == 0), stop=(i == NB - 1))
        for th in proj_pair_thunks(NCH - 1):
            th()
    nc.compile()
    return nc


def _to_mm_dtype(a):
    if MM_DTYPE == "bf16":
        import ml_dtypes
        return np.ascontiguousarray(a).astype(ml_dtypes.bfloat16)
    return np.ascontiguousarray(a).astype(np.float32)


def make_in_maps(X, W_qkv, W_proj, t_len=T):
    """Host-side sharding: slice + pre-arrange weights per head group,
    transpose X.  Layouts match the SBUF tensors so every weight DMA is
    fully contiguous:
      wq/wk/wv [128, 8, 256]: [p, kc, m] = W[kc*128+p, cols][m]
      wp       [128, 2, C]:   [64*hh+d, yc, m] = W_proj[(2*yc+hh)*64+d, m]
    """
    in_maps = []
    xts = [_to_mm_dtype(np.asarray(X[b, :t_len, :]).T) for b in range(B)]
    NKC = C // P
    for c in range(NCORES):
        b = c // (NCORES // B)
        h0 = HPC * (c % (NCORES // B))
        cols = slice(h0 * HD, (h0 + HPC) * HD)

        def warr(w):
            return _to_mm_dtype(
                np.ascontiguousarray(w).reshape(NKC, P, HPC * HD)
                .transpose(1, 0, 2))

        wp_c = np.ascontiguousarray(W_proj[cols, :])          # [256, C]
        wp2 = wp_c.reshape(2, 2, HD, C).transpose(1, 2, 0, 3).reshape(P, 2, C)
        in_maps.append({
            "xt": xts[b],
            "wq": warr(W_qkv[:, cols]),
            "wk": warr(W_qkv[:, C:][:, cols]),
            "wv": warr(W_qkv[:, 2 * C:][:, cols]),
            "wp": _to_mm_dtype(wp2),
        })
    return in_maps


_CACHE = {}
TRACE = False           # set True (e.g. from test.py) to capture an NTFF profile


def kernel(X, W_qkv, W_proj):
    import sys
    if "/opt/trn_rl_repo" not in sys.path:
        sys.path.insert(0, "/opt/trn_rl_repo")
    from concourse.bass_utils import run_bass_kernel_spmd

    X = np.asarray(X, dtype=np.float32)
    W_qkv = np.asarray(W_qkv, dtype=np.float32)
    W_proj = np.asarray(W_proj, dtype=np.float32)

    if "nc" not in _CACHE:
        _CACHE["nc"] = build_nc()
    nc = _CACHE["nc"]

    in_maps = make_in_maps(X, W_qkv, W_proj)
    res = run_bass_kernel_spmd(nc, in_maps, core_ids=list(range(NCORES)),
                               trace=TRACE)
    _CACHE["last"] = res
    out = np.empty((B, T, C), dtype=np.float32)
    ncb = NCORES // B
    for b in range(B):
        acc = res.results[b * ncb]["out"].astype(np.float32)
        for c in range(b * ncb + 1, (b + 1) * ncb):
            acc = acc + res.results[c]["out"].astype(np.float32)
        out[b] = acc
    return out


# revision 41
# speedup vs baseline: 1.1668x; 1.0010x over previous
"""Causal self-attention Trainium2 kernel (Bass/Tile), 8-core SPMD.

Problem: X[2, 2048, 1024], W_qkv[1024, 3072], W_proj[1024, 1024], H=16 heads.

Sharding: core c handles batch b = c // 4 and heads h0 = 4*(c % 4) .. h0+4
(tensor-parallel over heads + data-parallel over batch). Each core computes
a partial output  out_b = Y[:, heads] @ W_proj[head rows, :]  and the host
sums the 4 partials per batch (the W_proj row-shard reduction).

Per-core device layout ("transposed attention", no P transposes needed):
  Xt  [C, T]      X[b].T, all 32 [128, 512] tiles DMAed up front
  Qt,Kt [128,2,CH] per chunk, per head-pair group g: partition = 64*(h%2)+d
  V   [128,4,260] per chunk [token-block, head*65(+ones col)] for PV lhsT
  St  = Kt_blk.T @ Qt_chunk -> [keys 128, q 512] PSUM (K=d=64 contraction),
        both heads of a pair packed side-by-side in one [128, 1024] tile
  P   = exp(0.125*(St + causal_mask))  via ACT; ones-augmented PV gives
  Yt_aug = [V|1].T @ P -> [65, q 512]: rows 0-63 = Yt, row 64 = softmax sums

Scheduling: the TRN2 PE clock ramps 0.65 -> 1.2 -> 2.4 GHz with ~3us of
CONTINUOUS execution and drops back on idle, so the whole kernel is emitted
as one dense PE stream: warmup matmuls cover the initial weight DMAs, and
the QKV projection for token chunk tch+1 plus the output projection for
chunk qi-1 are injected one group per key-block iteration into the
attention loop over chunk qi, so the PE never starves while ACT runs exp.
ST matmuls and exp skip the fully-masked columns left of the causal
diagonal; PV reads only the surviving columns.
"""

import numpy as np
from collections import deque

B, T, C, H = 2, 2048, 1024, 16
HD = 64          # head dim
HPC = 4          # heads per core
P = 128
NCORES = 8
CH = 512         # token chunk (matmul free dim / q chunk)
KB = 128         # key block
MASK_VAL = -1.0e5
MM_DTYPE = "bf16"


def build_nc(t_len=T, mm_dtype=None):
    import concourse.bass as bass
    import concourse.mybir as mybir
    from concourse import bacc, library_config
    from concourse.tile import TileContext
    from contextlib import ExitStack

    mm_dtype = mm_dtype or MM_DTYPE
    f32 = mybir.dt.float32
    mdt = mybir.dt.bfloat16 if mm_dtype == "bf16" else mybir.dt.float32r
    Exp = mybir.ActivationFunctionType.Exp
    Alu = mybir.AluOpType

    NKC = C // P          # 8 contraction chunks over C
    NCH = t_len // CH     # token chunks
    NBC = CH // P         # token blocks per chunk (4)
    SC = 1.0 / np.sqrt(HD)

    nc = bacc.Bacc("TRN2", target_bir_lowering=False, debug=False,
                   num_devices=NCORES)

    xt_d = nc.dram_tensor("xt", [C, t_len], mdt, kind="ExternalInput").ap()
    wq_d = nc.dram_tensor("wq", [P, NKC, HPC * HD], mdt, kind="ExternalInput").ap()
    wk_d = nc.dram_tensor("wk", [P, NKC, HPC * HD], mdt, kind="ExternalInput").ap()
    wv_d = nc.dram_tensor("wv", [P, NKC, HPC * HD], mdt, kind="ExternalInput").ap()
    wp_d = nc.dram_tensor("wp", [P, 2, C], mdt, kind="ExternalInput").ap()
    out_d = nc.dram_tensor("out", [t_len, C], mdt, kind="ExternalOutput").ap()

    with TileContext(nc) as tc, ExitStack() as ctx:
        const = ctx.enter_context(tc.tile_pool(name="const", bufs=1))
        work = ctx.enter_context(tc.tile_pool(name="work", bufs=3))
        psm = ctx.enter_context(tc.tile_pool(name="psm", bufs=2, space="PSUM"))

        # ---- persistent SBUF tensors ----
        wq_sb = const.tile([P, NKC, HPC * HD], mdt, tag="wq")
        wk_sb = const.tile([P, NKC, HPC * HD], mdt, tag="wk")
        wv_sb = const.tile([P, NKC, HPC * HD], mdt, tag="wv")
        wp_sb = const.tile([P, 2, C], mdt, tag="wp")
        # per-chunk tensors (separate tiles -> no false cross-chunk deps)
        qt = [const.tile([P, 2, CH], mdt, tag=f"qt{t}", name=f"qt{t}")
              for t in range(NCH)]
        kt = [const.tile([P, 2, CH], mdt, tag=f"kt{t}", name=f"kt{t}")
              for t in range(NCH)]
        VE = HD + 8     # padded per-head V-block stride (ones, 64 dims, pad)
        va = [const.tile([P, NBC, HPC * VE], mdt, tag=f"va{t}",
                         name=f"va{t}") for t in range(NCH)]
        y2 = [const.tile([P, 2, CH], mdt, tag=f"y2{t}", name=f"y2{t}")
              for t in range(NCH)]
        xts = [[const.tile([P, CH], mdt, tag=f"xt{t}_{kc}", name=f"xt{t}_{kc}")
                for kc in range(NKC)] for t in range(NCH)]

        def xsrc(tch, kc, c0=0, c1=CH):
            return xts[tch][kc][:, c0:c1]
        warm = const.tile([P, CH], mdt, tag="warm")

        # ---- PE warmup: dense junk matmuls so the tensor engine p-state
        # ramps while the first weight/activation DMAs are in flight.  The
        # memset is the FIRST emitted instruction so nothing can precede it
        # in the DVE queue and stall the PE stream behind it. ----
        nc.vector.memset(warm[:], 0.0)
        wt = psm.tile([P, 2 * CH], f32, tag="st", bufs=3, name="warm")[:, :CH]
        NW = 18
        for i in range(NW):
            nc.tensor.matmul(wt[:], lhsT=warm[:, :P], rhs=warm[:],
                             start=(i == 0), stop=(i == NW - 1))
        nc.gpsimd.load_library(library_config.proxy)

        # ---- input DMAs: wq + chunk-0 Xt first so QKV(0) can start; wk/wv
        # issued early (Sync issues DMAs serially at ~0.6us each and the
        # k/v matmul groups need them before the Xt tail) ----
        nc.sync.dma_start(wq_sb[:, :NKC // 2], wq_d[:, :NKC // 2])
        nc.sync.dma_start(xts[0][0][:], xt_d[0:P, 0:CH])
        nc.sync.dma_start(wq_sb[:, NKC // 2:], wq_d[:, NKC // 2:])
        nc.sync.dma_start(xts[0][1][:], xt_d[P:2 * P, 0:CH])
        nc.sync.dma_start(wk_sb[:, :NKC // 2], wk_d[:, :NKC // 2])
        nc.sync.dma_start(wk_sb[:, NKC // 2:], wk_d[:, NKC // 2:])
        for kc in range(2, NKC):
            nc.sync.dma_start(xts[0][kc][:], xt_d[kc * P:(kc + 1) * P, 0:CH])
        nc.sync.dma_start(wv_sb[:, :NKC // 2], wv_d[:, :NKC // 2])
        nc.sync.dma_start(wv_sb[:, NKC // 2:], wv_d[:, NKC // 2:])
        nc.sync.dma_start(wp_sb[:], wp_d[:])
        for tch in range(1, NCH):
            for kc in range(NKC):
                nc.sync.dma_start(
                    xts[tch][kc][:],
                    xt_d[kc * P:(kc + 1) * P, tch * CH:(tch + 1) * CH])

        # ones columns of the augmented V (softmax denominator trick).  The
        # ones column comes FIRST in each head's 65-block so the PV sums row
        # lands in PSUM partition 0, where partition_broadcast can read it
        # without a bounce DMA.
        for t in range(NCH):
            for hl in range(HPC):
                nc.vector.memset(
                    va[t][:, :, hl * VE: hl * VE + 1], 1.0)

        # ---- QKV projection for one token chunk: 8 injectable PE groups ----
        def qkv_group_thunks(tch):
            thunks = []

            def qk_group(w_sb, dst, g):
                def run():
                    pq = psm.tile([P, 2 * CH], f32, tag="st", bufs=3,
                                  name="pq")[:, :CH]
                    for kc in range(NKC):
                        nc.tensor.matmul(
                            pq[:],
                            lhsT=w_sb[:, kc, g * P:(g + 1) * P],
                            rhs=xsrc(tch, kc),
                            start=(kc == 0), stop=(kc == NKC - 1))
                    nc.vector.tensor_copy(out=dst[tch][:, g, :], in_=pq[:])
                return run

            def v_group(vb):
                def run():
                    pv = psm.tile([P, 2 * CH], f32, tag="st", bufs=3,
                                  name="pv")[:, :HPC * HD]
                    for kc in range(NKC):
                        nc.tensor.matmul(
                            pv,
                            lhsT=xsrc(tch, kc, vb * P, (vb + 1) * P),
                            rhs=wv_sb[:, kc, :],
                            start=(kc == 0), stop=(kc == NKC - 1))
                    nc.vector.tensor_copy(
                        out=va[tch][:, vb, :]
                        .rearrange("p (h e) -> p h e", e=VE)[:, :, 1:HD + 1],
                        in_=pv.rearrange("p (h e) -> p h e", e=HD))
                return run

            for g in range(2):
                thunks.append(qk_group(wq_sb, qt, g))
                thunks.append(qk_group(wk_sb, kt, g))
            for vb in range(NBC):
                thunks.append(v_group(vb))
            return thunks

        # ---- output projection for one chunk: 8 injectable PE pairs ----
        def proj_pair_thunks(qi):
            thunks = []
            last = qi == NCH - 1

            def pair(tbl, oc):
                def run():
                    pp = psm.tile([P, 2 * CH], f32, tag="st", bufs=3,
                                  name="pp")[:, :CH]
                    for yc in range(2):
                        nc.tensor.matmul(
                            pp[:],
                            lhsT=y2[qi][:, yc, tbl * P:(tbl + 1) * P],
                            rhs=wp_sb[:, yc, oc * CH:(oc + 1) * CH],
                            start=(yc == 0), stop=(yc == 1))
                    ost = work.tile([P, CH], mdt, tag="ost", bufs=6)
                    # on the final chunk ACT is idle: alternate the PSUM
                    # drains between ACT and DVE so the pp slots free at
                    # twice the rate and the PE never waits on a drain
                    if last and oc == 0:
                        nc.scalar.copy(out=ost[:], in_=pp[:])
                    else:
                        nc.vector.tensor_copy(out=ost[:], in_=pp[:])
                    tb = qi * NBC + tbl
                    nc.sync.dma_start(
                        out_d[tb * P:(tb + 1) * P, oc * CH:(oc + 1) * CH],
                        ost[:])
                return run

            for tbl in range(NBC):
                for oc in range(2):
                    thunks.append(pair(tbl, oc))
            return thunks

        # ---- normalize Yt rows by the sums rows (ytp partition 0), store
        # into y2 via partition-shifting SBUF->SBUF DMAs.  Both heads of the
        # pair share one broadcast/reciprocal/multiply over [65, 2*CH]. ----
        def normalize_pair(g, qi, ytps, last=False):
            """The PSUM tiles are drained immediately (copies) so the pool
            slots free before the reciprocal chain completes.  On the final
            chunk the hh=1 copy moves to ACT (idle by then) so the two
            drains don't serialize on DVE ahead of the last proj."""
            bnc = work.tile([HD + 1, 2, CH], f32, tag="bounce", bufs=2)
            nc.vector.tensor_copy(out=bnc[:, 0, :], in_=ytps[0][:])
            if last:
                nc.scalar.copy(out=bnc[:, 1, :], in_=ytps[1][:])
            else:
                nc.vector.tensor_copy(out=bnc[:, 1, :], in_=ytps[1][:])
            # partition_broadcast / custom-DVE ops ignore the AP base
            # partition on HW; the sums rows are at partition 0 by layout.
            rb = work.tile([HD + 1, 2, CH], f32, tag="rb", bufs=2)
            nc.gpsimd.partition_broadcast(rb[:], bnc[0:1, :, :])
            nc.vector.reciprocal_approx_fast(out=rb[:], in_=rb[:])
            ybs = work.tile([HD + 1, 2, CH], mdt, tag="ybs", bufs=2)
            # row 0 computes sums * 1/sums; only rows 1..64 are stored.
            # (DVE partition base must be aligned, so operate on [0:65].)
            nc.vector.tensor_tensor(out=ybs[:], in0=bnc[:], in1=rb[:],
                                    op=Alu.mult)
            for hh in range(2):
                # shift each head's 64 rows to partitions hh*64 .. hh*64+63
                nc.sync.dma_start(y2[qi][hh * HD:(hh + 1) * HD, g, :],
                                  ybs[1:, hh, :])

        # ---- attention for one query chunk, with PE work injection ----
        def attention(qi, queue, start_at=0):
            nkb = NBC * (qi + 1)
            it = 0
            for g in range(2):
                ytps = [psm.tile([HD + 1, CH], f32, tag="yt", bufs=2,
                                 name=f"ytp{hh}") for hh in range(2)]

                def emit_pv(pts, kb, qoff):
                    for hh in range(2):
                        hl = 2 * g + hh
                        nc.tensor.matmul(
                            ytps[hh][:, qoff:],
                            lhsT=va[kb // NBC][:, kb % NBC,
                                               hl * VE:hl * VE + HD + 1],
                            rhs=pts[:, hh * CH + qoff:(hh + 1) * CH],
                            start=(kb == 0), stop=(kb == nkb - 1))

                prev = None
                for kb in range(nkb):
                    o4 = kb - NBC * qi          # 0..3 on the diagonal band
                    qoff = max(0, o4) * KB
                    stp = psm.tile([P, 2 * CH], f32, tag="st", bufs=3,
                                   name="stp")
                    for hh in range(2):
                        nc.tensor.matmul(
                            stp[:, hh * CH + qoff:(hh + 1) * CH],
                            lhsT=kt[kb // NBC][hh * HD:(hh + 1) * HD, g,
                                               (kb % NBC) * KB:(kb % NBC + 1) * KB],
                            rhs=qt[qi][hh * HD:(hh + 1) * HD, g, qoff:],
                            start=True, stop=True)
                    # PV of the previous block slots in while this block's
                    # exp runs, keeping PE fed (software pipelining).
                    if prev is not None:
                        emit_pv(*prev)
                    if queue and it >= start_at:
                        n = -(-len(queue) // max(1, 2 * nkb - it))
                        for _ in range(min(n, len(queue))):
                            queue.popleft()()
                    it += 1
                    pts = work.tile([P, 2 * CH], mdt, tag="p", bufs=4,
                                    name="pts")
                    if o4 >= 0:
                        nc.scalar.activation(
                            out=pts.rearrange("p (h c) -> p h c", h=2)[:, :, qoff:],
                            in_=stp.rearrange("p (h c) -> p h c", h=2)[:, :, qoff:],
                            func=Exp, scale=SC)
                        # causal mask: zero the upper triangle of the
                        # 128-wide diagonal band of P (keep where q >= k,
                        # i.e. band column f >= partition p).  Runs on
                        # GpSimd so the DVE queue never delays exp or PV.
                        for hh in range(2):
                            nc.gpsimd.affine_select(
                                out=pts[:, hh * CH + qoff:hh * CH + qoff + KB],
                                in_=pts[:, hh * CH + qoff:hh * CH + qoff + KB],
                                compare_op=Alu.is_ge, fill=0.0,
                                base=0, channel_multiplier=-1,
                                pattern=[[1, KB]])
                    else:
                        nc.scalar.activation(out=pts[:], in_=stp[:],
                                             func=Exp, scale=SC)
                    prev = (pts, kb, qoff)
                emit_pv(*prev)
                normalize_pair(g, qi, ytps,
                               last=(qi == NCH - 1 and g == 1))

        # ---- emission schedule ----
        def junk(n):
            """Dependency-free matmuls bridging a known PE stall so the
            p-state stays at max while DMAs land."""
            jt = psm.tile([P, 2 * CH], f32, tag="st", bufs=3,
                          name="junk")[:, :CH]
            for i in range(n):
                nc.tensor.matmul(jt[:], lhsT=warm[:, :P], rhs=warm[:],
                                 start=(i == 0), stop=(i == n - 1))

        # phase A: only q-g0/k-g0 of chunk 0 up front; the v-groups and the
        # g1 projections are injected into attention(0) just ahead of need.
        # The initial weight/Xt DMAs land slower than the PE consumes, so
        # junk bridges keep the clock hot between the first groups.
        t0 = qkv_group_thunks(0)
        t0[0]()
        junk(8)
        t0[1]()
        junk(8)
        for qi in range(NCH):
            queue = deque()
            if qi == 0:
                queue.extend([t0[4], t0[5], t0[2], t0[3], t0[6], t0[7]])
            if qi + 1 < NCH:
                queue.extend(qkv_group_thunks(qi + 1))
            start_at = 0
            if qi > 0:
                queue.extend(proj_pair_thunks(qi - 1))
                start_at = 5 if qi == NCH - 1 else 0
            attention(qi, queue, start_at)
            while queue:
                queue.popleft()()
        # bridge: junk matmuls keep the PE p-state at max while the last
        # chunk's normalize chains run, so the final proj starts hot and
        # immediately.
        wt2 = psm.tile([P, 2 * CH], f32, tag="st", bufs=3,
                       name="bridge")[:, :CH]
        NB = 34
        for i in range(NB):
            nc.tensor.matmul(wt2[:], lhsT=warm[:, :P], rhs=warm[:],
                             start=(i == 0), stop=(i == NB - 1))
        for th in proj_pair_thunks(NCH - 1):
            th()
    nc.compile()
    return nc


def _to_mm_dtype(a):
    if MM_DTYPE == "bf16":
        import ml_dtypes
        return np.ascontiguousarray(a).astype(ml_dtypes.bfloat16)
    return np.ascontiguousarray(a).astype(np.float32)


def make_in_maps(X, W_qkv, W_proj, t_len=T):
    """Host-side sharding: slice + pre-arrange weights per head group,
    transpose X.  Layouts match the SBUF tensors so every weight DMA is
    fully contiguous:
      wq/wk/wv [128, 8, 256]: [p, kc, m] = W[kc*128+p, cols][m]
      wp       [128, 2, C]:   [64*hh+d, yc, m] = W_proj[(2*yc+hh)*64+d, m]
    """
    in_maps = []
    xts = [_to_mm_dtype(np.asarray(X[b, :t_len, :]).T) for b in range(B)]
    NKC = C // P
    for c in range(NCORES):
        b = c // (NCORES // B)
        h0 = HPC * (c % (NCORES // B))
        cols = slice(h0 * HD, (h0 + HPC) * HD)

        def warr(w):
            return _to_mm_dtype(
                np.ascontiguousarray(w).reshape(NKC, P, HPC * HD)
                .transpose(1, 0, 2))

        wp_c = np.ascontiguousarray(W_proj[cols, :])          # [256, C]
        wp2 = wp_c.reshape(2, 2, HD, C).transpose(1, 2, 0, 3).reshape(P, 2, C)
        in_maps.append({
            "xt": xts[b],
            "wq": warr(W_qkv[:, cols]),
            "wk": warr(W_qkv[:, C:][:, cols]),
            "wv": warr(W_qkv[:, 2 * C:][:, cols]),
            "wp": _to_mm_dtype(wp2),
        })
    return in_maps


_CACHE = {}
TRACE = False           # set True (e.g. from test.py) to capture an NTFF profile


def kernel(X, W_qkv, W_proj):
    import sys
    if "/opt/trn_rl_repo" not in sys.path:
        sys.path.insert(0, "/opt/trn_rl_repo")
    from concourse.bass_utils import run_bass_kernel_spmd

    X = np.asarray(X, dtype=np.float32)
    W_qkv = np.asarray(W_qkv, dtype=np.float32)
    W_proj = np.asarray(W_proj, dtype=np.float32)

    if "nc" not in _CACHE:
        _CACHE["nc"] = build_nc()
    nc = _CACHE["nc"]

    in_maps = make_in_maps(X, W_qkv, W_proj)
    res = run_bass_kernel_spmd(nc, in_maps, core_ids=list(range(NCORES)),
                               trace=TRACE)
    _CACHE["last"] = res
    out = np.empty((B, T, C), dtype=np.float32)
    ncb = NCORES // B
    for b in range(B):
        acc = res.results[b * ncb]["out"].astype(np.float32)
        for c in range(b * ncb + 1, (b + 1) * ncb):
            acc = acc + res.results[c]["out"].astype(np.float32)
        out[b] = acc
    return out
